# revision 22
# baseline (speedup 1.0000x reference)
"""Binarized-MLP (BNN) kernel for Trainium2, data-parallel over batch on 8 NeuronCores.

Reference computation:
    h      = x @ sign(W1) + b1          x:[8192,4096] W1:[4096,512]
    logits = sign(h) @ sign(W2) + b2    W2:[512,10]
    out    = softmax(logits)            [8192,10]

Device strategy (per core, batch shard of 1024 rows):
  - x is supplied pre-transposed and split hi/lo in bf16 (x = hi + lo to
    ~2^-18 relative accuracy), so the dominant matmul runs as two bf16
    TensorE passes accumulating into the same fp32 PSUM bank — fp32-grade
    accuracy at bf16 speed.
  - Layout: stationary = sign(W1) f-tile [128f x 128j], moving = xT f-tile
    [128f x 512b] -> PSUM [128j x 512b]; all 8 PSUM banks hold the full
    per-core h [512 x 1024] and accumulate across the 32 f-tiles.
  - Inputs are host-packed so four f-tiles arrive per DMA with 8KB
    contiguous per partition line (DMA issue cost here is per-descriptor:
    ~0.6us per 128-partition DMA regardless of bytes).
  - The last four f-tiles run bank-major so each PSUM bank finishes early
    and sign(h)/second-matmul/softmax overlap the remaining big matmuls.
  - sign(h)+b1 is fused into one ScalarE Sign-activation (bias=b1) straight
    out of PSUM into bf16 SBUF tiles, laid out [j, b] as the stationary
    operand of the second matmul. sign() of weights adds a +1e-30 bias so
    sign(0) == +1 like the reference's where(x >= 0) (W1 contains one 0.0).
  - Second matmul: stationary = sign(h) [128j x 128b], moving = sign(W2)
    [128j x 10] accumulated over 4 j-tiles -> PSUM [128b x 10].
  - Softmax on [128b, 10] tiles: add b2 (host-replicated [128,10]),
    reduce_max(negate) -> Exp activation with per-row bias and fused row-sum
    (accum_out), reciprocal, scale into a collect tile, single packed DMA out.
"""

import numpy as np
import ml_dtypes

import concourse.bass as bass
import concourse.tile as tile
from concourse import mybir
from concourse.bass_utils import run_bass_kernel_spmd
from bass_rust import ScopedClock, VectorClock

BF16 = mybir.dt.bfloat16
F32 = mybir.dt.float32

B, F, H, C = 8192, 4096, 512, 10
NCORES = 8
BC = B // NCORES          # 1024 batch rows per core
NF = F // 128             # 32 f-tiles (contraction)
NJ = H // 128             # 4 j-tiles (hidden)
NBC = BC // 512           # 2 moving-operand chunks of 512
NBT = BC // 128           # 8 output b-tiles
NQ = NF // 4              # 8 quads of f-tiles (4 per DMA)


class _PatchedTileContext(tile.TileContext):
    """Workaround for the walrus build in this container only accepting one
    sem wait on a CTRL-type (Drain) instruction: spread the exit drain's
    per-proc waits across several drains with one wait each."""

    def _drain_and_barrier(self, tick_clock, wait_clock):
        gc = tick_clock.global_clock
        ticks = list(gc)
        nprocs = len(ticks)
        engines = [
            self.nc.sync,
            self.nc.gpsimd,
            self.nc.vector,
            self.nc.scalar,
            self.nc.tensor,
        ]
        # Cheap wait-carriers: one engine NOP per pending proc tick, spread
        # round-robin so the waits resolve in parallel (a DRAIN costs ~1us on
        # some engines; a NOP ~50ns).
        k = 0
        for i, t in enumerate(ticks):
            if t == 0:
                continue
            partial = [0] * nprocs
            partial[i] = t
            inst = engines[k % len(engines)].nop()
            k += 1
            wait_clock.add_sem_waits(
                inst.ins, ScopedClock({None: VectorClock(partial)})
            )
        self.nc.sync.drain()

        self.nc.all_engine_barrier()
        assert self.sems is not None
        popped = self.nc._tile_sem_poison_stack.pop()
        assert popped is self._sem_poison
        # gpsimd-only cleanup (range-clear is a single op there); no closing
        # barrier — each engine halts after its own stream, and NEFF
        # completion waits for all engines anyway.
        self.nc.clear_and_free_semaphores(list(self.sems.allocated().values()))


def _split_waits_json(raw: bytes) -> bytes:
    """The walrus build in this container accepts at most ONE sem wait per
    instruction (bass's own wait_op asserts the same). Tile attaches several.
    Rewrite the serialized BIR: excess waits become standalone EventSemaphore
    wait instructions on the same engine immediately before the instruction —
    semantically identical, since the engine blocks there first."""
    import json as _json

    m = _json.loads(raw)
    ctr = 0
    for fn in m.get("functions", []):
        for bb in fn.get("blocks", []):
            insts = bb.get("instructions", [])
            new_insts = []
            for inst in insts:
                si = inst.get("sync_info")
                waits = si.get("on_wait") or [] if si else []
                if len(waits) > 1:
                    for w in waits[:-1]:
                        new_insts.append(
                            {
                                "debug": inst.get("debug", 0),
                                "engine": inst["engine"],
                                "ins": [],
                                "outs": [],
                                "name": f"WSPLIT-{ctr}",
                                "opcode": "EventSemaphore",
                                "sync_info": {"on_update": [], "on_wait": [w]},
                            }
                        )
                        ctr += 1
                    si["on_wait"] = [waits[-1]]
                new_insts.append(inst)
            bb["instructions"] = new_insts
    return _json.dumps(m).encode()


def _install_wait_splitter(nc: bass.Bass) -> None:
    orig = nc.to_json_bytes

    def patched():
        return _split_waits_json(orig())

    nc.to_json_bytes = patched


def build_kernel() -> bass.Bass:
    nc = bass.Bass()
    # Quad-packed streams: row q*128+p holds 4 f-subtiles contiguously.
    # xtq sub-layout per row: [i=0..3][hi 1024 | lo 1024]  (8KB / partition line)
    xtq = nc.dram_tensor("xtq", [NQ * 128, 4 * 2 * BC], BF16, kind="ExternalInput")
    # w1q sub-layout per row: [i=0..3][512 h-cols]          (4KB / partition line)
    w1q = nc.dram_tensor("w1q", [NQ * 128, 4 * H], BF16, kind="ExternalInput")
    b1p = nc.dram_tensor("b1p", [128, NJ], F32, kind="ExternalInput")
    w2p = nc.dram_tensor("w2p", [128, NJ * C], F32, kind="ExternalInput")
    b2r = nc.dram_tensor("b2r", [128, C], F32, kind="ExternalInput")
    out = nc.dram_tensor("out", [BC, C], F32, kind="ExternalOutput")

    with _PatchedTileContext(nc) as tc:
        with (
            tc.tile_pool(name="consts", bufs=1) as consts,
            tc.tile_pool(name="w1raw", bufs=2) as w1raw_pool,
            tc.tile_pool(name="w1s", bufs=2) as w1s_pool,
            tc.tile_pool(name="xin", bufs=12) as xin_pool,
            tc.tile_pool(name="signh", bufs=NJ * NBC) as signh_pool,
            tc.tile_pool(name="psum", bufs=8, space="PSUM") as psum_pool,
            tc.tile_pool(name="smx", bufs=4) as smx_pool,
        ):
            tiny = consts.tile([128, 1], F32, name="tiny", tag="tiny")
            nc.vector.memset(tiny[:], 1e-30)

            psumB = [
                [psum_pool.tile([128, 512], F32, name="psB", tag="psB") for _ in range(NBC)]
                for _ in range(NJ)
            ]

            def quad_in(q):
                raw = w1raw_pool.tile([128, 2048], BF16, name="w1raw", tag="w1raw")
                nc.sync.dma_start(raw[:], w1q[q * 128:(q + 1) * 128, :])
                w1s = w1s_pool.tile([128, 2048], BF16, name="w1s", tag="w1s")
                for i in range(4):
                    nc.scalar.sign(
                        w1s[:, i * 512:(i + 1) * 512],
                        raw[:, i * 512:(i + 1) * 512],
                        bias=tiny[:],
                    )
                xfs = []
                for i in range(4):
                    xf = xin_pool.tile([128, 2048], BF16, name="xin", tag="xin")
                    nc.sync.dma_start(
                        xf[:], xtq[q * 128:(q + 1) * 128, i * 2048:(i + 1) * 2048]
                    )
                    xfs.append(xf)
                return w1s, xfs

            def quad_mms(w1s, xfs, i, j, bc, start, stop):
                lhs = w1s[:, i * 512 + j * 128:i * 512 + (j + 1) * 128]
                xf = xfs[i]
                hi = xf[:, bc * 512:(bc + 1) * 512]
                lo = xf[:, 1024 + bc * 512:1024 + (bc + 1) * 512]
                nc.tensor.matmul(psumB[j][bc][:], lhs, hi, start=start, stop=False)
                nc.tensor.matmul(psumB[j][bc][:], lhs, lo, start=False, stop=stop)

            # ---- phase 1: quads 0..NQ-2, f-major over all 8 banks ----
            b1_t = w2raw = w2s = b2_t = None
            for q in range(NQ - 1):
                if q == 0:
                    with tc.high_priority():
                        w1s, xf = quad_in(q)
                else:
                    w1s, xf = quad_in(q)
                if q == 0:
                    # constants: packed, one DMA each, after the first quad's
                    # stream DMAs so they stay off the startup critical path
                    b1_t = consts.tile([128, NJ], F32, name="b1t", tag="b1t")
                    nc.sync.dma_start(b1_t[:], b1p[:, :])
                    w2raw = consts.tile([128, NJ * C], F32, name="w2raw", tag="w2raw")
                    nc.sync.dma_start(w2raw[:], w2p[:, :])
                    b2_t = consts.tile([128, C], F32, name="b2", tag="b2")
                    nc.sync.dma_start(b2_t[:], b2r[:, :])
                for i in range(4):
                    for j in range(NJ):
                        for bc in range(NBC):
                            quad_mms(w1s, xf, i, j, bc,
                                     start=(q == 0 and i == 0), stop=False)

            # ---- phase 2: last quad bank-major; sign/mm2/softmax overlap ----
            w1s_l, xf_l = quad_in(NQ - 1)
            w2s = consts.tile([128, NJ * C], BF16, name="w2s", tag="w2s")
            nc.scalar.sign(w2s[:], w2raw[:], bias=tiny[:])
            signh = [[None] * NBC for _ in range(NJ)]
            collect = smx_pool.tile([128, NBT * C], F32, name="collect", tag="collect")
            for bc in range(NBC):
                for j in range(NJ):
                    for i in range(4):
                        quad_mms(w1s_l, xf_l, i, j, bc,
                                 start=False, stop=(i == 3))
                    s = signh_pool.tile([128, 512], BF16, name="signh", tag="signh")
                    nc.scalar.sign(s[:], psumB[j][bc][:], bias=b1_t[:, j:j + 1])
                    signh[j][bc] = s
                for bt in range(bc * 4, bc * 4 + 4):
                    col = (bt % 4) * 128
                    ps2 = psum_pool.tile([128, C], F32, name="psD", tag="psB")
                    for j in range(NJ):
                        nc.tensor.matmul(
                            ps2[:],
                            signh[j][bc][:, col:col + 128],
                            w2s[:, j * C:(j + 1) * C],
                            start=(j == 0),
                            stop=(j == NJ - 1),
                        )
                    logits = smx_pool.tile([128, C], F32, name="logits", tag="logits")
                    nc.vector.tensor_add(logits[:], ps2[:], b2_t[:])
                    negmax = smx_pool.tile([128, 1], F32, name="negmax", tag="negmax")
                    nc.vector.reduce_max(
                        negmax[:], logits[:], axis=mybir.AxisListType.X, negate=True
                    )
                    e = smx_pool.tile([128, C], F32, name="e", tag="e")
                    ssum = smx_pool.tile([128, 1], F32, name="ssum", tag="ssum")
                    nc.scalar.activation(
                        e[:],
                        logits[:],
                        mybir.ActivationFunctionType.Exp,
                        bias=negmax[:],
                        accum_out=ssum[:],
                    )
                    lns = smx_pool.tile([128, 1], F32, name="lns", tag="lns")
                    nc.scalar.activation(
                        lns[:], ssum[:], mybir.ActivationFunctionType.Ln
                    )
                    negms = smx_pool.tile([128, 1], F32, name="negms", tag="negms")
                    nc.vector.tensor_sub(negms[:], negmax[:], lns[:])
                    nc.scalar.activation(
                        collect[:, bt * C:(bt + 1) * C],
                        logits[:],
                        mybir.ActivationFunctionType.Exp,
                        bias=negms[:],
                    )

            # single packed output DMA: out[bt*128+p, c] = collect[p, bt*10+c]
            nc.sync.dma_start(
                out.rearrange("(bt p) c -> p bt c", p=128),
                collect[:].rearrange("p (bt c) -> p bt c", c=C),
            )

    _install_wait_splitter(nc)
    return nc


_cached_nc = None


def _get_nc() -> bass.Bass:
    global _cached_nc
    if _cached_nc is None:
        _cached_nc = build_kernel()
    return _cached_nc


def kernel(inputs, W1, b1, W2, b2):
    x = np.ascontiguousarray(np.asarray(inputs, dtype=np.float32))
    W1 = np.asarray(W1, dtype=np.float32)
    b1 = np.asarray(b1, dtype=np.float32)
    W2 = np.asarray(W2, dtype=np.float32)
    b2 = np.asarray(b2, dtype=np.float32)

    w1_bf = W1.astype(ml_dtypes.bfloat16)
    # [4096, 512] -> quad-packed [NQ*128, 4*512]
    w1_pack = np.ascontiguousarray(
        w1_bf.reshape(NQ, 4, 128, H).transpose(0, 2, 1, 3).reshape(NQ * 128, 4 * H)
    )
    b1_pack = np.ascontiguousarray(b1.reshape(NJ, 128).T)
    w2_pack = np.ascontiguousarray(
        W2.reshape(NJ, 128, C).transpose(1, 0, 2).reshape(128, NJ * C)
    )
    b2_rep = np.ascontiguousarray(np.broadcast_to(b2.reshape(1, C), (128, C)))

    in_maps = []
    for c in range(NCORES):
        xc_t = x[c * BC:(c + 1) * BC, :].T  # [F, BC]
        hi = xc_t.astype(ml_dtypes.bfloat16)
        lo = (xc_t - hi.astype(np.float32)).astype(ml_dtypes.bfloat16)
        pack = np.empty((NQ, 128, 4, 2, BC), dtype=ml_dtypes.bfloat16)
        pack[:, :, :, 0] = hi.reshape(NQ, 4, 128, BC).transpose(0, 2, 1, 3)
        pack[:, :, :, 1] = lo.reshape(NQ, 4, 128, BC).transpose(0, 2, 1, 3)
        in_maps.append(
            {
                "xtq": pack.reshape(NQ * 128, 4 * 2 * BC),
                "w1q": w1_pack,
                "w2p": w2_pack,
                "b1p": b1_pack,
                "b2r": b2_rep,
            }
        )

    nc = _get_nc()
    res = run_bass_kernel_spmd(nc, in_maps, core_ids=list(range(NCORES)))
    global last_results
    last_results = res
    out = np.concatenate([res.results[c]["out"] for c in range(NCORES)], axis=0)
    return out.astype(np.float32)


last_results = None
